# revision 3
# baseline (speedup 1.0000x reference)
"""ConvLSTM-PSLA Trainium2 kernel (8 NeuronCores).

Strategy:
  - Attention path (corr/softmax/apply/u-convs/update_feat) is spatially
    sharded across the 8 cores (each core computes its slice of B*H rows,
    with halos provided by the host). fp32 elementwise on VectorE.
  - update_feat slices are AllGathered (bf16) on-device.
  - The big 3x3 gate convs (xw/hw, ~98.5% of FLOPs) are tensor-parallel:
    each core computes C/8 channels of each of the 4 LSTM gates (gate-aligned
    channel packing => the LSTM elementwise needs no cross-core comms).
  - hconv (doesn't depend on update_feat) overlaps the attention + AllGather;
    its partial result bounces through DRAM (bf16). xconv accumulates on top,
    then the LSTM gate math runs per 512-pixel chunk.
"""

import sys

for _p in ("/opt/trn_rl_repo", "/opt/trn_rl_repo/concourse"):
    if _p not in sys.path:
        sys.path.insert(0, _p)

import numpy as np
import ml_dtypes

import concourse.bass as bass
import concourse.bacc as bacc
import concourse.mybir as mybir
import concourse.tile as tile

R = 8
B = 2
BF = mybir.dt.bfloat16
F32 = mybir.dt.float32
NPBF = ml_dtypes.bfloat16
MULT = mybir.AluOpType.mult
ADD = mybir.AluOpType.add
AF = mybir.ActivationFunctionType


def _lv(C, H):
    W = H
    Hp, Wp = H + 2, W + 2
    G = C // R
    lv = dict(
        C=C, H=H, W=W, Hp=Hp, Wp=Wp, G=G,
        KC=C // 128,
        M4=4 * G,
        NM=(4 * G) // 128,
        NPIX=B * Hp * Wp,
        RO=(B * H) // R,          # own rows per core
    )
    lv["WIN"] = lv["RO"] + 2      # attention window rows
    lv["NW"] = lv["WIN"] * Wp     # window free size
    lv["NXP"] = (lv["WIN"] + 2) * Wp + 2  # xp slice incl. +-1 col margin
    lv["NOWN"] = lv["RO"] * Wp
    lv["TOFF"] = Wp + 1
    lv["GFREE"] = lv["NPIX"] + 2 * lv["TOFF"]
    # gate-conv output chunking over padded pixels
    ch, s = [], 0
    while s < lv["NPIX"]:
        n = min(512, lv["NPIX"] - s)
        ch.append((s, n))
        s += n
    lv["GCH"] = ch
    # attention window chunking (<=512, >=... for psum)
    lv["WCH"] = _chunks(lv["NW"])
    lv["OCH"] = _chunks(lv["NOWN"])
    # own rows start per rank, in padded-flat coords
    ps = []
    for k in range(R):
        row = k * lv["RO"]
        b, h0 = row // H, row % H
        ps.append(b * Hp * Wp + (h0 + 1) * Wp)
    lv["PSTART"] = ps
    # taps
    lv["TAPS"] = [(dy, dx) for dy in (-1, 0, 1) for dx in (-1, 0, 1)]
    return lv


def _chunks(n):
    k = (n + 511) // 512
    base = n // k
    out, s = [], 0
    for i in range(k):
        sz = base + (1 if i < n - base * k else 0)
        out.append((s, sz))
        s += sz
    assert s == n
    return out


LEVELS_REAL = [_lv(512, 64), _lv(1024, 32)]


# --------------------------------------------------------------------------
# program builder (SPMD, one program for all 8 cores)
# --------------------------------------------------------------------------

def build_program(levels):
    nc = bacc.Bacc("TRN2", target_bir_lowering=False, debug=False, num_devices=R)

    ins, outs = {}, {}

    def di(name, shape, dt):
        ins[name] = nc.dram_tensor(name, list(shape), dt, kind="ExternalInput").ap()
        return ins[name]

    def do(name, shape, dt):
        outs[name] = nc.dram_tensor(name, list(shape), dt, kind="ExternalOutput").ap()
        return outs[name]

    for l, lv in enumerate(levels):
        C, M4, NW, NXP, NOWN, NPIX, GFREE, G = (
            lv["C"], lv["M4"], lv["NW"], lv["NXP"], lv["NOWN"], lv["NPIX"],
            lv["GFREE"], lv["G"])
        di(f"wtx{l}", (9, C, M4), BF)
        di(f"wth{l}", (9, C, M4), BF)
        di(f"bias{l}", (M4,), F32)
        di(f"hpad{l}", (C, GFREE), BF)
        di(f"cpre{l}", (G, NPIX), F32)
        di(f"xps{l}", (C, NXP), F32)
        di(f"xsl{l}", (C, NW), F32)
        di(f"msk{l}", (1, NW), F32)
        di(f"u1wt{l}", (2 * C, 256), BF)
        di(f"u2wt{l}", (9, 256, 16), BF)
        di(f"u3wt{l}", (16, 2), BF)
        di(f"u1b{l}", (256,), F32)
        di(f"u2b{l}", (16,), F32)
        di(f"u3b{l}", (2,), F32)
        do(f"nf{l}", (C, NOWN), F32)
        do(f"hn{l}", (G, NPIX), F32)
        do(f"cn{l}", (G, NPIX), F32)

    with tile.TileContext(nc) as tc:
        _emit(nc, tc, levels, ins, outs)
    nc.compile()
    return nc


def _emit(nc, tc, levels, ins, outs):
    es = tc.alloc_tile_pool(name="const", bufs=1)
    ones128 = es.tile([128, 128], F32, name="ones128")
    nc.vector.memset(ones128[:, :], 1.0)
    ones1 = es.tile([1, 128], F32, name="ones1")
    nc.vector.memset(ones1[:, :], 1.0)

    # DRAM scratch (AllGather bounce + cxh bounce)
    dp = tc.alloc_tile_pool(name="dram", bufs=1, space="DRAM")
    ag_in, ag_out, cxh_d = [], [], []
    for l, lv in enumerate(levels):
        ag_in.append(dp.tile([lv["C"], lv["NOWN"]], BF, name=f"agin{l}"))
        ag_out.append(dp.tile([R * lv["C"], lv["NOWN"]], BF,
                              addr_space="Shared", name=f"agout{l}"))
        cxh_d.append(dp.tile([lv["M4"], lv["NPIX"]], BF, name=f"cxhd{l}"))

    ap_psum = tc.alloc_tile_pool(name="apsum", bufs=3, space="PSUM")

    # ---------------- attention (both levels), emitted first -------------
    for l, lv in enumerate(levels):
        _attention(nc, tc, lv, l, ins, outs, ag_in, ag_out, ones128, ones1,
                   ap_psum, es)
    ap_psum.release()

    # ---------------- hconv (both levels) --------------------------------
    h_psum = tc.alloc_tile_pool(name="hpsum", bufs=3, space="PSUM")
    for l, lv in enumerate(levels):
        _hconv(nc, tc, lv, l, ins, cxh_d, h_psum)
    h_psum.release()

    # ---------------- xconv + gates (both levels) ------------------------
    x_psum = tc.alloc_tile_pool(name="xpsum", bufs=6, space="PSUM")
    for l, lv in enumerate(levels):
        _xconv_gates(nc, tc, lv, l, ins, outs, ag_out, cxh_d, x_psum)
    x_psum.release()
    dp.release()
    es.release()


def _attention(nc, tc, lv, l, ins, outs, ag_in, ag_out, ones128, ones1,
               psum, es):
    C, KC, NW, NXP, NOWN, Wp = lv["C"], lv["KC"], lv["NW"], lv["NXP"], lv["NOWN"], lv["Wp"]
    TAPS, WCH, OCH = lv["TAPS"], lv["WCH"], lv["OCH"]
    XO = 1 + Wp  # xps tile offset of window position 0

    outer = tc.alloc_tile_pool(name=f"attn{l}o", bufs=1)
    xsl = [outer.tile([128, NW], F32, name=f"xsl{l}_{c}") for c in range(KC)]
    acc = [outer.tile([128, NW], F32, name=f"acc{l}_{c}") for c in range(KC)]
    xps = [outer.tile([128, NXP], F32, name=f"xps{l}_{c}") for c in range(KC)]
    for c in range(KC):
        nc.sync.dma_start(xsl[c][:, :], ins[f"xsl{l}"][c * 128:(c + 1) * 128, :])
        nc.sync.dma_start(xps[c][:, :], ins[f"xps{l}"][c * 128:(c + 1) * 128, :])
    msk = outer.tile([1, NW], F32, name=f"msk{l}")
    nc.sync.dma_start(msk[:, :], ins[f"msk{l}"][:, :])

    # ---- corr + exp -----------------------------------------------------
    p1 = tc.alloc_tile_pool(name=f"attn{l}a", bufs=1)
    expk = [p1.tile([1, NW], F32, name=f"exp{l}_{t}") for t in range(9)]
    prods = tc.alloc_tile_pool(name=f"attn{l}pr", bufs=KC + 1)
    for t, (dy, dx) in enumerate(TAPS):
        d = dy * Wp + dx
        pr = []
        for c in range(KC):
            p = prods.tile([128, NW], F32, name=f"prod{l}", tag="prod")
            nc.vector.tensor_tensor(p[:, :], xsl[c][:, :],
                                    xps[c][:, XO + d:XO + d + NW], MULT)
            pr.append(p)
        for (s, n) in WCH:
            ps = psum.tile([128, 512], F32, name=f"cps{l}", tag="aps")
            for c in range(KC):
                nc.tensor.matmul(ps[:, :n], ones128[:, :], pr[c][:, s:s + n],
                                 start=(c == 0), stop=(c == KC - 1))
            nc.scalar.activation(expk[t][:, s:s + n], ps[0:1, :n], AF.Exp)
    prods.release()

    # ---- softmax denominator -> normalizer row --------------------------
    ssum = p1.tile([1, NW], F32, name=f"ssum{l}")
    nc.vector.tensor_tensor(ssum[:, :], expk[0][:, :], expk[1][:, :], ADD)
    for t in range(2, 9):
        nc.vector.tensor_tensor(ssum[:, :], ssum[:, :], expk[t][:, :], ADD)
    rs = p1.tile([1, NW], F32, name=f"rs{l}")
    nc.vector.reciprocal(rs[:, :], ssum[:, :])
    nc.vector.tensor_tensor(rs[:, :], rs[:, :], msk[:, :], MULT)

    # ---- apply: acc[c] = sum_t exp_t * shift_t(xp), then * (mask/sum) ---
    ebcp = tc.alloc_tile_pool(name=f"attn{l}eb", bufs=2)
    tmpp = tc.alloc_tile_pool(name=f"attn{l}tm", bufs=2)
    for t, (dy, dx) in enumerate(TAPS):
        d = dy * Wp + dx
        eb = ebcp.tile([128, NW], F32, name=f"ebc{l}", tag="ebc")
        for (s, n) in WCH:
            ps = psum.tile([128, 512], F32, name=f"ebp{l}", tag="aps")
            nc.tensor.matmul(ps[:, :n], ones1[:, :], expk[t][:, s:s + n],
                             start=True, stop=True)
            nc.scalar.activation(eb[:, s:s + n], ps[:, :n], AF.Copy)
        for c in range(KC):
            if t == 0:
                nc.vector.tensor_tensor(acc[c][:, :], eb[:, :],
                                        xps[c][:, XO + d:XO + d + NW], MULT)
            else:
                tm = tmpp.tile([128, NW], F32, name=f"atm{l}", tag="atm")
                nc.vector.tensor_tensor(tm[:, :], eb[:, :],
                                        xps[c][:, XO + d:XO + d + NW], MULT)
                nc.vector.tensor_tensor(acc[c][:, :], acc[c][:, :], tm[:, :], ADD)
    # normalize+mask (broadcast rs)
    nm = ebcp.tile([128, NW], F32, name=f"nmsk{l}", tag="ebc")
    for (s, n) in WCH:
        ps = psum.tile([128, 512], F32, name=f"nmp{l}", tag="aps")
        nc.tensor.matmul(ps[:, :n], ones1[:, :], rs[:, s:s + n],
                         start=True, stop=True)
        nc.scalar.activation(nm[:, s:s + n], ps[:, :n], AF.Copy)
    for c in range(KC):
        nc.vector.tensor_tensor(acc[c][:, :], acc[c][:, :], nm[:, :], MULT)
    tmpp.release()
    ebcp.release()
    p1.release()

    # ---- u-convs --------------------------------------------------------
    p2 = tc.alloc_tile_pool(name=f"attn{l}b", bufs=1)
    nfb = [p2.tile([128, NW], BF, name=f"nfb{l}_{c}") for c in range(KC)]
    xbf = [p2.tile([128, NW], BF, name=f"xbf{l}_{c}") for c in range(KC)]
    for c in range(KC):
        nc.vector.tensor_copy(nfb[c][:, :], acc[c][:, :])
        nc.vector.tensor_copy(xbf[c][:, :], xsl[c][:, :])

    u1w = [p2.tile([128, 256], BF, name=f"u1w{l}_{c}") for c in range(2 * KC)]
    for c in range(2 * KC):
        nc.sync.dma_start(u1w[c][:, :], ins[f"u1wt{l}"][c * 128:(c + 1) * 128, :])
    u1bt = p2.tile([128, 2], F32, name=f"u1bt{l}")
    for m in range(2):
        nc.sync.dma_start(u1bt[:, m:m + 1], ins[f"u1b{l}"][m * 128:(m + 1) * 128])
    u2w = [p2.tile([128, 16], BF, name=f"u2w{l}_{t}_{c}")
           for t in range(9) for c in range(2)]
    for t in range(9):
        for c in range(2):
            nc.sync.dma_start(u2w[t * 2 + c][:, :],
                              ins[f"u2wt{l}"][t, c * 128:(c + 1) * 128, :])
    u2bt = p2.tile([16, 1], F32, name=f"u2bt{l}")
    nc.sync.dma_start(u2bt[:, :], ins[f"u2b{l}"][:])
    u3w = p2.tile([16, 2], BF, name=f"u3w{l}")
    nc.sync.dma_start(u3w[:, :], ins[f"u3wt{l}"][:, :])
    u3bt = p2.tile([1, 2], F32, name=f"u3bt{l}")
    nc.sync.dma_start(u3bt[0:1, :], ins[f"u3b{l}"][:])

    # u1: (256, NW) over window, into padded-by-1-col tiles
    u1s = [p2.tile([128, NW + 2], BF, name=f"u1s{l}_{m}") for m in range(2)]
    for m in range(2):
        nc.vector.memset(u1s[m][:, 0:1], 0.0)
        nc.vector.memset(u1s[m][:, NW + 1:NW + 2], 0.0)
        for (s, n) in WCH:
            ps = psum.tile([128, 512], F32, name=f"u1p{l}", tag="aps")
            for c2 in range(2 * KC):
                rhs = nfb[c2] if c2 < KC else xbf[c2 - KC]
                nc.tensor.matmul(ps[:, :n], u1w[c2][:, m * 128:(m + 1) * 128],
                                 rhs[:, s:s + n],
                                 start=(c2 == 0), stop=(c2 == 2 * KC - 1))
            nc.scalar.activation(u1s[m][:, 1 + s:1 + s + n], ps[:, :n],
                                 AF.Identity, bias=u1bt[:, m:m + 1])

    # u2 (3x3, 256->16) over own rows only
    u2s = p2.tile([16, NOWN], BF, name=f"u2s{l}")
    for (s, n) in OCH:
        ps = psum.tile([128, 512], F32, name=f"u2p{l}", tag="aps")
        first = True
        for t, (dy, dx) in enumerate(TAPS):
            d = dy * Wp + dx
            for m in range(2):
                nc.tensor.matmul(ps[0:16, :n], u2w[t * 2 + m][:, :],
                                 u1s[m][:, 1 + Wp + s + d:1 + Wp + s + d + n],
                                 start=first, stop=(t == 8 and m == 1))
                first = False
        nc.scalar.activation(u2s[:, s:s + n], ps[0:16, :n], AF.Identity,
                             bias=u2bt[:, :])

    # u3 (1x1, 16->2) -> two (1, NOWN) sigmoid rows
    uw0 = p2.tile([1, NOWN], F32, name=f"uw0{l}")
    uw1 = p2.tile([1, NOWN], F32, name=f"uw1{l}")
    for (s, n) in OCH:
        for j, uw in enumerate((uw0, uw1)):
            ps = psum.tile([128, 512], F32, name=f"u3p{l}", tag="aps")
            nc.tensor.matmul(ps[0:1, :n], u3w[:, j:j + 1], u2s[:, s:s + n],
                             start=True, stop=True)
            nc.scalar.activation(uw[:, s:s + n], ps[0:1, :n], AF.Sigmoid,
                                 bias=u3bt[0:1, j:j + 1])
    den = p2.tile([1, NOWN], F32, name=f"uden{l}")
    nc.vector.tensor_tensor(den[:, :], uw0[:, :], uw1[:, :], ADD)
    nc.vector.reciprocal(den[:, :], den[:, :])
    nc.vector.tensor_tensor(uw0[:, :], uw0[:, :], den[:, :], MULT)
    nc.vector.tensor_tensor(uw1[:, :], uw1[:, :], den[:, :], MULT)
    ub0 = p2.tile([128, NOWN], F32, name=f"ub0{l}")
    ub1 = p2.tile([128, NOWN], F32, name=f"ub1{l}")
    for j, (src, dst) in enumerate(((uw0, ub0), (uw1, ub1))):
        for (s, n) in OCH:
            ps = psum.tile([128, 512], F32, name=f"ubp{l}", tag="aps")
            nc.tensor.matmul(ps[:, :n], ones1[:, :], src[:, s:s + n],
                             start=True, stop=True)
            nc.scalar.activation(dst[:, s:s + n], ps[:, :n], AF.Copy)

    # ---- update_feat on own rows; nf output + AG bounce -----------------
    for c in range(KC):
        upf = p2.tile([128, NOWN], F32, name=f"upf{l}", tag="upf", bufs=2)
        tm2 = p2.tile([128, NOWN], F32, name=f"upt{l}", tag="upt", bufs=2)
        nc.vector.tensor_tensor(upf[:, :], acc[c][:, Wp:Wp + NOWN], ub0[:, :], MULT)
        nc.vector.tensor_tensor(tm2[:, :], xsl[c][:, Wp:Wp + NOWN], ub1[:, :], MULT)
        nc.vector.tensor_tensor(upf[:, :], upf[:, :], tm2[:, :], ADD)
        nc.sync.dma_start(outs[f"nf{l}"][c * 128:(c + 1) * 128, :], upf[:, :])
        upb = p2.tile([128, NOWN], BF, name=f"upb{l}", tag="upb", bufs=2)
        nc.vector.tensor_copy(upb[:, :], upf[:, :])
        nc.sync.dma_start(ag_in[l][c * 128:(c + 1) * 128, :], upb[:, :])

    nc.gpsimd.collective_compute(
        "AllGather", mybir.AluOpType.bypass,
        replica_groups=[list(range(R))],
        ins=[ag_in[l].opt()], outs=[ag_out[l].opt()],
    )
    p2.release()
    outer.release()


def _hconv(nc, tc, lv, l, ins, cxh_d, psum):
    C, KC, NM, M4, Wp, TOFF, GFREE = (lv["C"], lv["KC"], lv["NM"], lv["M4"],
                                      lv["Wp"], lv["TOFF"], lv["GFREE"])
    pool = tc.alloc_tile_pool(name=f"hc{l}", bufs=1)
    w = [pool.tile([128, M4], BF, name=f"wh{l}_{t}_{c}")
         for t in range(9) for c in range(KC)]
    for t in range(9):
        for c in range(KC):
            nc.sync.dma_start(w[t * KC + c][:, :],
                              ins[f"wth{l}"][t, c * 128:(c + 1) * 128, :])
    h = [pool.tile([128, GFREE], BF, name=f"hh{l}_{c}") for c in range(KC)]
    for c in range(KC):
        nc.sync.dma_start(h[c][:, :], ins[f"hpad{l}"][c * 128:(c + 1) * 128, :])
    ev = tc.alloc_tile_pool(name=f"hce{l}", bufs=4)
    for (s, n) in lv["GCH"]:
        for m in range(NM):
            ps = psum.tile([128, 512], F32, name=f"hps{l}", tag="hps")
            first = True
            for c in range(KC):
                for t, (dy, dx) in enumerate(lv["TAPS"]):
                    d = dy * Wp + dx
                    nc.tensor.matmul(
                        ps[:, :n], w[t * KC + c][:, m * 128:(m + 1) * 128],
                        h[c][:, TOFF + s + d:TOFF + s + d + n],
                        start=first, stop=(c == KC - 1 and t == 8))
                    first = False
            sb = ev.tile([128, 512], BF, name=f"hev{l}", tag="hev")
            nc.scalar.activation(sb[:, :n], ps[:, :n], AF.Copy)
            nc.sync.dma_start(cxh_d[l][m * 128:(m + 1) * 128, s:s + n], sb[:, :n])
    ev.release()
    pool.release()


def _xconv_gates(nc, tc, lv, l, ins, outs, ag_out, cxh_d, psum):
    C, KC, NM, M4, G, Wp = lv["C"], lv["KC"], lv["NM"], lv["M4"], lv["G"], lv["Wp"]
    TOFF, GFREE, NPIX, NOWN = lv["TOFF"], lv["GFREE"], lv["NPIX"], lv["NOWN"]
    Hp = lv["Hp"]
    pool = tc.alloc_tile_pool(name=f"xc{l}", bufs=1)
    w = [pool.tile([128, M4], BF, name=f"wx{l}_{t}_{c}")
         for t in range(9) for c in range(KC)]
    for t in range(9):
        for c in range(KC):
            nc.sync.dma_start(w[t * KC + c][:, :],
                              ins[f"wtx{l}"][t, c * 128:(c + 1) * 128, :])
    bt = pool.tile([128, NM], F32, name=f"bt{l}")
    for m in range(NM):
        nc.sync.dma_start(bt[:, m:m + 1], ins[f"bias{l}"][m * 128:(m + 1) * 128])
    ct = pool.tile([128, NPIX], F32, name=f"ct{l}")
    if G == 64:
        nc.sync.dma_start(ct[64:128, :], ins[f"cpre{l}"][:, :])
    else:
        nc.sync.dma_start(ct[:, :], ins[f"cpre{l}"][:, :])

    # gathered update_feat, placed into padded-flat layout
    u = [pool.tile([128, GFREE], BF, name=f"ug{l}_{c}") for c in range(KC)]
    for c in range(KC):
        nc.vector.memset(u[c][:, 0:TOFF], 0.0)
        nc.vector.memset(u[c][:, TOFF + NPIX:GFREE], 0.0)
        # zero the two pad rows of each image (not covered by any rank slice)
        z = u[c][:, TOFF:TOFF + NPIX].rearrange("p (b h w) -> p b h w",
                                                b=B, h=Hp, w=Wp)
        nc.vector.memset(z[:, :, 0, :], 0.0)
        nc.vector.memset(z[:, :, Hp - 1, :], 0.0)
        for r in range(R):
            nc.sync.dma_start(
                u[c][:, TOFF + lv["PSTART"][r]:TOFF + lv["PSTART"][r] + NOWN],
                ag_out[l][r * C + c * 128:r * C + (c + 1) * 128, :])

    ev = tc.alloc_tile_pool(name=f"xg{l}", bufs=3)
    cx = tc.alloc_tile_pool(name=f"cxl{l}", bufs=4)
    for (s, n) in lv["GCH"]:
        pss = []
        for m in range(NM):
            ps = psum.tile([128, 512], F32, name=f"xps{l}", tag="xps")
            first = True
            for c in range(KC):
                for t, (dy, dx) in enumerate(lv["TAPS"]):
                    d = dy * Wp + dx
                    nc.tensor.matmul(
                        ps[:, :n], w[t * KC + c][:, m * 128:(m + 1) * 128],
                        u[c][:, TOFF + s + d:TOFF + s + d + n],
                        start=first, stop=(c == KC - 1 and t == 8))
                    first = False
            pss.append(ps)
        # add hconv partial (streamed from DRAM) -> t tiles
        ts = []
        for m in range(NM):
            cxt = cx.tile([128, 512], BF, name=f"cxt{l}", tag="cxt")
            nc.sync.dma_start(cxt[:, :n], cxh_d[l][m * 128:(m + 1) * 128, s:s + n])
            tt = ev.tile([128, 512], F32, name=f"tt{l}_{m}", tag=f"tt{m}")
            nc.vector.tensor_tensor(tt[:, :n], pss[m][:, :n], cxt[:, :n], ADD)
            ts.append(tt)
        if G == 64:
            _gates_half(nc, ev, ts, bt, ct, outs, l, s, n)
        else:
            _gates_full(nc, ev, ts, bt, ct, outs, l, s, n)
    cx.release()
    ev.release()
    pool.release()


def _gates_half(nc, ev, ts, bt, ct, outs, l, s, n):
    # M-chunk0 = [i;o], M-chunk1 = [g;f]
    t0, t1 = ts
    s0 = ev.tile([128, 512], F32, name=f"s0{l}", tag="g_s0")
    nc.scalar.activation(s0[:, :n], t0[:, :n], AF.Sigmoid, bias=bt[:, 0:1])
    gf = ev.tile([128, 512], F32, name=f"gf{l}", tag="g_gf")
    nc.scalar.activation(gf[0:64, :n], t1[0:64, :n], AF.Tanh, bias=bt[0:64, 1:2])
    nc.scalar.activation(gf[64:128, :n], t1[64:128, :n], AF.Sigmoid,
                         bias=bt[64:128, 1:2])
    m2 = ev.tile([128, 512], F32, name=f"m2{l}", tag="g_m2")
    nc.vector.tensor_tensor(m2[0:64, :n], s0[0:64, :n], gf[0:64, :n], MULT)
    mv = ev.tile([128, 512], F32, name=f"mv{l}", tag="g_mv")
    nc.sync.dma_start(mv[64:128, :n], m2[0:64, :n])
    cn = ev.tile([128, 512], F32, name=f"cnt{l}", tag="g_cn")
    nc.vector.tensor_tensor(cn[64:128, :n], gf[64:128, :n], ct[64:128, s:s + n],
                            MULT)
    nc.vector.tensor_tensor(cn[64:128, :n], cn[64:128, :n], mv[64:128, :n], ADD)
    nc.sync.dma_start(outs[f"cn{l}"][:, s:s + n], cn[64:128, :n])
    tc2 = ev.tile([128, 512], F32, name=f"tc2{l}", tag="g_tc")
    nc.scalar.activation(tc2[64:128, :n], cn[64:128, :n], AF.Tanh)
    hn = ev.tile([128, 512], F32, name=f"hnt{l}", tag="g_hn")
    nc.vector.tensor_tensor(hn[64:128, :n], s0[64:128, :n], tc2[64:128, :n], MULT)
    nc.sync.dma_start(outs[f"hn{l}"][:, s:s + n], hn[64:128, :n])


def _gates_full(nc, ev, ts, bt, ct, outs, l, s, n):
    # M-chunks = [i, f, o, g]
    ti, tf, to, tg = ts
    i2 = ev.tile([128, 512], F32, name=f"i2{l}", tag="g_i2")
    nc.scalar.activation(i2[:, :n], ti[:, :n], AF.Sigmoid, bias=bt[:, 0:1])
    f2 = ev.tile([128, 512], F32, name=f"f2{l}", tag="g_f2")
    nc.scalar.activation(f2[:, :n], tf[:, :n], AF.Sigmoid, bias=bt[:, 1:2])
    o2 = ev.tile([128, 512], F32, name=f"o2{l}", tag="g_o2")
    nc.scalar.activation(o2[:, :n], to[:, :n], AF.Sigmoid, bias=bt[:, 2:3])
    g2 = ev.tile([128, 512], F32, name=f"g2{l}", tag="g_g2")
    nc.scalar.activation(g2[:, :n], tg[:, :n], AF.Tanh, bias=bt[:, 3:4])
    cn = ev.tile([128, 512], F32, name=f"cnt{l}", tag="g_cn")
    nc.vector.tensor_tensor(cn[:, :n], f2[:, :n], ct[:, s:s + n], MULT)
    m2 = ev.tile([128, 512], F32, name=f"m2{l}", tag="g_m2")
    nc.vector.tensor_tensor(m2[:, :n], i2[:, :n], g2[:, :n], MULT)
    nc.vector.tensor_tensor(cn[:, :n], cn[:, :n], m2[:, :n], ADD)
    nc.sync.dma_start(outs[f"cn{l}"][:, s:s + n], cn[:, :n])
    tc2 = ev.tile([128, 512], F32, name=f"tc2{l}", tag="g_tc")
    nc.scalar.activation(tc2[:, :n], cn[:, :n], AF.Tanh)
    hn = ev.tile([128, 512], F32, name=f"hnt{l}", tag="g_hn")
    nc.vector.tensor_tensor(hn[:, :n], o2[:, :n], tc2[:, :n], MULT)
    nc.sync.dma_start(outs[f"hn{l}"][:, s:s + n], hn[:, :n])


# --------------------------------------------------------------------------
# host side
# --------------------------------------------------------------------------

def _pad_bhw(x):
    """(B,C,H,W) -> (C, B*(H+2), W+2) zero-padded, fp32."""
    Bn, C, H, W = x.shape
    p = np.zeros((C, Bn, H + 2, W + 2), np.float32)
    p[:, :, 1:H + 1, 1:W + 1] = np.transpose(x, (1, 0, 2, 3))
    return p.reshape(C, Bn * (H + 2), W + 2)


def _perm(lv):
    C, G = lv["C"], lv["G"]
    perms = []
    for k in range(R):
        j = np.arange(G) + k * G
        if G == 64:
            p = np.concatenate([0 * C + j, 2 * C + j, 3 * C + j, 1 * C + j])
        else:
            p = np.concatenate([0 * C + j, 1 * C + j, 2 * C + j, 3 * C + j])
        perms.append(p)
    return perms


def host_prepare(levels, inputs):
    """Build per-core in_maps from the full inputs dict."""
    maps = [dict() for _ in range(R)]
    for l, lv in enumerate(levels):
        C, H, W, Hp, Wp, G = lv["C"], lv["H"], lv["W"], lv["Hp"], lv["Wp"], lv["G"]
        KC, NW, NXP, WIN, RO = lv["KC"], lv["NW"], lv["NXP"], lv["WIN"], lv["RO"]
        xw = np.asarray(inputs[f"xw{l}"], np.float32)
        hw = np.asarray(inputs[f"hw{l}"], np.float32)
        xb = np.asarray(inputs[f"xb{l}"], np.float32)
        hb = np.asarray(inputs[f"hb{l}"], np.float32)
        perms = _perm(lv)
        wtx_all = np.ascontiguousarray(xw.transpose(2, 3, 1, 0)).reshape(9, C, 4 * C)
        wth_all = np.ascontiguousarray(hw.transpose(2, 3, 1, 0)).reshape(9, C, 4 * C)
        bias_all = xb + hb
        hpadf = _pad_bhw(np.asarray(inputs[f"h{l}"], np.float32)).reshape(C, -1)
        hg = np.zeros((C, lv["GFREE"]), NPBF)
        hg[:, lv["TOFF"]:lv["TOFF"] + lv["NPIX"]] = hpadf.astype(NPBF)
        cpad = _pad_bhw(np.asarray(inputs[f"c{l}"], np.float32)).reshape(C, -1)
        xpp = _pad_bhw(np.asarray(inputs[f"xp{l}"], np.float32))  # (C, B*Hp, Wp)
        xx = _pad_bhw(np.asarray(inputs[f"x{l}"], np.float32))
        nrows = B * Hp

        u1wt = np.ascontiguousarray(
            np.asarray(inputs[f"u1w{l}"], np.float32)[:, :, 0, 0].T).astype(NPBF)
        u2wt = np.ascontiguousarray(
            np.asarray(inputs[f"u2w{l}"], np.float32).transpose(2, 3, 1, 0)
        ).reshape(9, 256, 16).astype(NPBF)
        u3wt = np.ascontiguousarray(
            np.asarray(inputs[f"u3w{l}"], np.float32)[:, :, 0, 0].T).astype(NPBF)

        for k in range(R):
            m = maps[k]
            m[f"wtx{l}"] = np.ascontiguousarray(wtx_all[:, :, perms[k]]).astype(NPBF)
            m[f"wth{l}"] = np.ascontiguousarray(wth_all[:, :, perms[k]]).astype(NPBF)
            m[f"bias{l}"] = np.ascontiguousarray(bias_all[perms[k]])
            m[f"hpad{l}"] = hg
            m[f"cpre{l}"] = np.ascontiguousarray(cpad[k * G:(k + 1) * G, :])
            row = k * RO
            b, h0 = row // H, row % H
            pr = b * Hp + h0 + 1
            # xp rows [pr-2, pr+RO+2), x rows [pr-1, pr+RO+1), zero-clipped
            xps = np.zeros((C, WIN + 2, Wp), np.float32)
            lo, hi = pr - 2, pr + RO + 2
            clo, chi = max(lo, 0), min(hi, nrows)
            xps[:, clo - lo:chi - lo, :] = xpp[:, clo:chi, :]
            xpf = np.zeros((C, NXP), np.float32)
            xpf[:, 1:1 + (WIN + 2) * Wp] = xps.reshape(C, -1)
            m[f"xps{l}"] = xpf
            xsl = np.zeros((C, WIN, Wp), np.float32)
            lo2, hi2 = pr - 1, pr + RO + 1
            cl2, ch2 = max(lo2, 0), min(hi2, nrows)
            xsl[:, cl2 - lo2:ch2 - lo2, :] = xx[:, cl2:ch2, :]
            m[f"xsl{l}"] = xsl.reshape(C, NW)
            # window validity mask (interior pixels only)
            gm = np.zeros((1, WIN, Wp), np.float32)
            for r in range(WIN):
                gr = pr - 1 + r
                if 0 <= gr < nrows and 1 <= (gr % Hp) <= H:
                    gm[0, r, 1:W + 1] = 1.0
            m[f"msk{l}"] = gm.reshape(1, NW)
            m[f"u1wt{l}"] = u1wt
            m[f"u2wt{l}"] = u2wt
            m[f"u3wt{l}"] = u3wt
            m[f"u1b{l}"] = np.asarray(inputs[f"u1b{l}"], np.float32)
            m[f"u2b{l}"] = np.asarray(inputs[f"u2b{l}"], np.float32)
            m[f"u3b{l}"] = np.asarray(inputs[f"u3b{l}"], np.float32)
    return maps


def assemble_outputs(levels, results):
    """Per-core result dicts -> reference-shaped output tuple."""
    out = []
    for what in ("hn", "cn"):
        for l, lv in enumerate(levels):
            C, H, W, Hp, Wp, G = (lv["C"], lv["H"], lv["W"], lv["Hp"], lv["Wp"],
                                  lv["G"])
            full = np.zeros((B, C, H, W), np.float32)
            for k in range(R):
                v = results[k][f"{what}{l}"].reshape(G, B, Hp, Wp)
                full[:, k * G:(k + 1) * G] = np.transpose(
                    v[:, :, 1:H + 1, 1:W + 1], (1, 0, 2, 3))
            out.append(full)
    nfs = []
    for l, lv in enumerate(levels):
        C, H, W, Wp, RO = lv["C"], lv["H"], lv["W"], lv["Wp"], lv["RO"]
        rows = np.zeros((C, B * H, W), np.float32)
        for k in range(R):
            v = results[k][f"nf{l}"].reshape(C, RO, Wp)
            rows[:, k * RO:(k + 1) * RO, :] = v[:, :, 1:W + 1]
        nfs.append(np.transpose(rows.reshape(C, B, H, W), (1, 0, 2, 3)))
    # reference returns (hn0, cn0, hn1, cn1, nf0, nf1)
    return out[0], out[2], out[1], out[3], nfs[0], nfs[1]


_PROG = None


def _get_prog():
    global _PROG
    if _PROG is None:
        _PROG = build_program(LEVELS_REAL)
    return _PROG


def kernel(**inputs):
    # map reference input names to level-indexed names
    ren = {}
    for l in range(2):
        for nm in ("h", "c", "xp", "x", "u1w", "u1b", "u2w", "u2b", "u3w",
                   "u3b", "xw", "xb", "hw", "hb"):
            ren[f"{nm}{l}"] = inputs[f"{nm}{l}"]
    from concourse.bass_utils import run_bass_kernel_spmd
    nc = _get_prog()
    maps = host_prepare(LEVELS_REAL, ren)
    res = run_bass_kernel_spmd(nc, maps, core_ids=list(range(R)))
    return assemble_outputs(LEVELS_REAL, res.results)
